# revision 15
# baseline (speedup 1.0000x reference)
"""Trainium2 Bass kernel for the 16-qubit angle-encoder (nn_Encoder).

Math: out[b, k] = (1/256) * exp(i * sum_q s_q(k) * pi * x[b, q]) where
s_q(k) = +1 if bit (15-q) of k is set else -1.  Split k = hi*256 + lo:
each output row is a complex outer product of a 256-entry U table and a
256-entry W table.  Each core handles 32 batch rows (data parallel).

The run is HBM-store-bound: 16 MiB of complex64 output per core vs 96 KB
of table input, so the kernel is organized to keep the store DMA stream
saturated from the earliest possible moment:

- host precomputes the per-row tables (256*768 sins per core, ~1/700th
  of the output work) in float64 and ships them bf16 in K-major layout:
  row r in {0,1}, cols b*256+hi hold U_r[b,hi]/256, cols 8192+b*512+n
  hold W_r[b,n] with n=2*lo+c.  U_0/W_0 carry cos phases, U_1/W_1 sin
  phases arranged so out = U_0^T W_0 + U_1^T W_1 directly produces
  re/im-interleaved rows (2e-3 rel err vs the 2e-2 gate).
- one 96 KB input DMA (SP ring), then per (b, chunk) block: a K=2 bf16
  matmul [2,128]x[2,512] -> PSUM [128,512], a PSUM->SBUF copy
  (alternating DVE/ACT so neither is the bottleneck), and a 256 KiB
  contiguous store on the SP ring.  The store stream hits the per-core
  DMA roofline (~360 B/ns) with no table-build phase to hide.
"""

import sys

sys.path.insert(0, "/opt/trn_rl_repo")

import numpy as np
import ml_dtypes

BF16 = ml_dtypes.bfloat16
N_QUBITS = 16
BATCH = 256
N_CORES = 8
B_PER_CORE = BATCH // N_CORES  # 32
PI = float(np.pi)

_COMPILED = {}


def _sign_base() -> np.ndarray:
    j = np.arange(256)
    q = np.arange(8)[:, None]
    return (2.0 * ((j >> (7 - q)) & 1) - 1.0).astype(np.float64)


def _tables_input(xs: np.ndarray) -> np.ndarray:
    """[2, B*768] bf16 tables, K-major, per-b interleaved: row r cols
    b*768+hi hold U_r[b,hi]/256, cols b*768+256+n hold W_r[b,n], n=2lo+c."""
    B = B_PER_CORE
    s8 = _sign_base()  # [8, 256]
    x = xs.astype(np.float64)
    ph = (PI * x[:, 0:8]) @ s8  # [B, 256]
    pl = (PI * x[:, 8:16]) @ s8  # [B, 256]
    n = np.arange(512)
    lo = n >> 1
    c = n & 1
    t = np.zeros((2, B, 768), np.float64)
    for r in range(2):
        # U_r[b, hi] = sin(ph + pi/2*(1-r)) / 256   (r=0: cos, r=1: sin)
        t[r, :, 0:256] = np.sin(ph + (PI / 2) * (1 - r)) / 256.0
        # W_r[b, 2lo+c] = sin(pl[lo] + pi/2*(1+r) - pi/2*c)
        t[r, :, 256:768] = np.sin(
            pl[:, lo] + (PI / 2) * (1 + r) - (PI / 2) * c)
    return t.reshape(2, B * 768).astype(BF16)


def _build_module(n_rep: int = 1, full_rep: bool = False):
    import concourse.bacc as bacc
    import concourse.tile as tile
    import concourse.mybir as mybir

    fp32 = mybir.dt.float32
    bf16 = mybir.dt.bfloat16

    nc = bacc.Bacc("TRN2", target_bir_lowering=False, debug=False,
                   num_devices=N_CORES)
    B = B_PER_CORE
    t_in = nc.declare_dram_parameter("t0", [2, B * 768], bf16, isOutput=False)
    # [b, chunk, hi_in_chunk, 2*lo+c] f32 == row-major [b, 65536] complex64
    y_out = nc.declare_dram_parameter("y", [B, 2, 128, 512], fp32,
                                      isOutput=True)
    w0 = B * 256

    with tile.TileContext(nc) as tc:
        with (
            tc.tile_pool(name="tables", bufs=1) as tp,
            tc.tile_pool(name="stage", bufs=10) as sp,
            tc.tile_pool(name="psum", bufs=7, space="PSUM") as pp,
        ):
            t0 = tp.tile([2, B * 768], bf16)
            nc.sync.dma_start(t0[:], t_in[:])

            def emit_stream(rep):
                for b in range(B):
                    w_rhs = t0[0:2, b * 768 + 256:b * 768 + 768]
                    for chunk in range(2):
                        ps = pp.tile([128, 512], fp32)
                        nc.tensor.matmul(
                            ps[:],
                            t0[0:2, b * 768 + chunk * 128:
                               b * 768 + (chunk + 1) * 128],
                            w_rhs, start=True, stop=True)
                        st = sp.tile([128, 512], fp32, tag="st")
                        if (b * 2 + chunk) % 2 == 0:
                            nc.vector.tensor_copy(st[:], ps[:])
                        else:
                            nc.scalar.copy(st[:], ps[:])
                        nc.sync.dma_start(y_out[b, chunk], st[:])

            for rep in range(n_rep):
                emit_stream(rep)

    nc.compile()
    return nc


def _get_compiled(n_rep: int = 1, full_rep: bool = False):
    key = ("nc", n_rep, full_rep)
    if key not in _COMPILED:
        _COMPILED[key] = _build_module(n_rep, full_rep)
    return _COMPILED[key]


def _make_inputs(x: np.ndarray) -> list:
    return [
        {"t0": _tables_input(x[c * B_PER_CORE:(c + 1) * B_PER_CORE])}
        for c in range(N_CORES)
    ]


def _run(inputs: np.ndarray, trace: bool = False):
    from concourse.bass_utils import run_bass_kernel_spmd

    nc = _get_compiled()
    x = np.asarray(inputs, dtype=np.float32)
    assert x.shape == (BATCH, N_QUBITS)
    in_maps = _make_inputs(x)
    res = run_bass_kernel_spmd(nc, in_maps, core_ids=list(range(N_CORES)),
                               trace=trace)
    parts = []
    for c in range(N_CORES):
        y = np.ascontiguousarray(res.results[c]["y"])  # [32, 2, 128, 512] f32
        parts.append(y.reshape(B_PER_CORE, 2 ** N_QUBITS * 2).view(np.complex64))
    out = np.concatenate(parts, axis=0)
    return out, res


def kernel(inputs: np.ndarray) -> np.ndarray:
    out, _ = _run(inputs, trace=False)
    return out
